# revision 32
# baseline (speedup 1.0000x reference)
"""GCNBlock (GCNConv + LayerNorm + LeakyReLU + residual) on 8 TRN2 NeuronCores.

Strategy (graph/data parallel over destination nodes, streaming device
kernel at the memory roofline):
  * 64-node destination "windows" are assigned to cores (greedy-balanced)
    and PAIRED into 49 virtual 128-row slots per core so the epilogue runs
    at full 128-partition width.
  * Host does structure/layout prep only: degrees, the edge order (grouped
    by core/slot/half, padded to 128-edge tiles), per-edge message rows
    msgs = 8*dinv[src]*dinv[dst]*x[src] in fp8e3m4 (linearity: the full
    symmetric normalization is folded into the message values; the 8x
    prescale keeps e3m4 out of its subnormal range and is divided back out
    of W), and per-tile one-hot destination matrices (fp8e3m4, exact 0/1).
    Self-loops ride along as ordinary edges.
  * Device (all FLOPs of the reference): per 128-edge tile the PE
    accumulates aggT[c, j64] += msgs[e, c]^T @ S[e, j64] into the
    [C, 4, 2, 64] PSUM group tile of 4 virtual windows; per group the
    epilogue does (aggT)^T @ (W/8) + ones^T b (bias via rank-1 matmul into
    PSUM), LN stats via E[t^2]-mu^2 (3D strided reduces), a fused
    per-window affine (t*rstd - mu*rstd) on the Scalar engine, LeakyReLU
    via max(z, 0.01 z), and the x residual — spread across Scalar, Vector
    and GpSimd so everything hides under the DMA stream.
  * Everything streams sequentially from HBM (no per-row descriptors):
    ~19.5 MB/core total traffic (fp8 messages + 64-wide fp8 one-hots).

kernel(**inputs) takes the FULL inputs and returns the FULL [N, C] output.
"""

import os

import numpy as np

N = 50000
E = 600000
C = 128
P = 128
WIN = 64  # destination window width (one-hot width)
NCORES = 8
NW64 = (N + WIN - 1) // WIN  # 782 destination windows
HALVES = (NW64 + NCORES - 1) // NCORES  # 98 windows per core
SLOTS = (HALVES + 1) // 2  # 49 virtual (paired) windows per core
GRP = 4  # virtual windows per epilogue group / PSUM bank
LN_EPS = 1e-5
ALPHA = 0.01
MSG_SCALE = 8.0  # fp8e3m4 prescale; divided back out of W
CH = 64  # tiles (of 128 edges) per streamed chunk
CH0 = 16  # small first chunk so the PE starts early
OSPAN = 4  # slots per output store
EPI_DELAY = 8  # next-group tiles issued before a group's deferred epilogue

_CACHE: dict = {}
LAST_RESULT = None


# --------------------------------------------------------------------------
# Host-side sharding / layout prep (structure only + fp8/fp16 copies)
# --------------------------------------------------------------------------
def _host_prep(x, edge_index):
    import ml_dtypes

    f8e3 = ml_dtypes.float8_e3m4

    src = np.asarray(edge_index[0], dtype=np.int64)
    dst = np.asarray(edge_index[1], dtype=np.int64)

    deg = (np.bincount(dst, minlength=N) + 1.0).astype(np.float64)
    dinv = (1.0 / np.sqrt(deg)).astype(np.float32)

    nodes = np.arange(N, dtype=np.int64)
    asrc = np.concatenate([src, nodes])
    adst = np.concatenate([dst, nodes])
    win = adst // WIN

    cnt = np.bincount(win, minlength=NW64)  # edges (incl self-loops) per window

    # greedy balanced assignment of 64-windows to cores (largest first)
    order = np.argsort(-cnt, kind="stable")
    loads = np.zeros(NCORES, np.int64)
    nwins = np.zeros(NCORES, np.int64)
    core_of_win = np.full(NW64, -1, np.int64)
    for w in order:
        cand = np.where(nwins < HALVES)[0]
        c = cand[np.argmin(loads[cand])]
        core_of_win[w] = c
        loads[c] += cnt[w]
        nwins[c] += 1

    # half assignment: windows within a core sorted by size desc; half index
    # h -> slot j = h//2, half = h%2.  Keeps per-(slot,half) max-over-cores
    # tile caps tight.
    slot_wins = np.full((NCORES, SLOTS, 2), -1, np.int64)
    half_of_win = np.zeros(NW64, np.int64)  # flat half index within core
    for c in range(NCORES):
        ws = sorted(np.where(core_of_win == c)[0], key=lambda w: -cnt[w])
        for h, w in enumerate(ws):
            slot_wins[c, h // 2, h % 2] = w
            half_of_win[w] = h

    # per-(slot,half) tile capacity (shared across cores); >=1 so empty
    # halves still get one all-zero tile (keeps PSUM regions honestly
    # written, never read-before-write)
    cap = np.ones((SLOTS, 2), np.int64)
    for c in range(NCORES):
        for j in range(SLOTS):
            for h in range(2):
                w = slot_wins[c, j, h]
                if w >= 0:
                    cap[j, h] = max(cap[j, h], (cnt[w] + P - 1) // P)
    capf = cap.reshape(-1)
    T = int(capf.sum())
    tile_off = (np.cumsum(capf) - capf).reshape(SLOTS, 2)

    # flat destination position for every augmented edge
    ecore = core_of_win[win]
    ehalf = half_of_win[win]
    key = ecore * (2 * SLOTS) + ehalf
    sidx = np.argsort(key, kind="stable")
    key_s = key[sidx]
    uniq, start = np.unique(key_s, return_index=True)
    within = np.arange(key_s.size, dtype=np.int64) - start[
        np.searchsorted(uniq, key_s)
    ]
    half_s = key_s % (2 * SLOTS)
    slot_s = half_s // 2
    h_s = half_s % 2
    tile = tile_off[slot_s, h_s] + (within >> 7)  # tile index within core
    prow = within & 127  # partition (edge slot within tile)
    core_s = key_s // (2 * SLOTS)
    drel = (adst[sidx] % WIN).astype(np.int64)

    # fp8 message rows: full symmetric normalization folded in, 8x prescale
    es = sidx  # augmented-edge ids in layout order
    mvals = x[asrc[es]] * (
        (MSG_SCALE * dinv[asrc[es]] * dinv[adst[es]])[:, None]
    )
    msgs = np.zeros((NCORES, P, T, C), f8e3)
    msgs[core_s, prow, tile] = mvals.astype(f8e3)
    msgs_pre = np.ascontiguousarray(msgs).reshape(NCORES, P, T * C)

    # per-(partition, tile) destination row for on-device one-hot generation
    # (-1 on padded slots -> all-zero one-hot row, exactly)
    drelw = np.full((NCORES, P, T), -1.0, np.float16)
    drelw[core_s, prow, tile] = drel.astype(np.float16)

    # residual x windows: partition p of slot j = node winA*64+p (p<64)
    # or winB*64+(p-64); padded rows -> 0
    xwin_pre = np.zeros((NCORES, P, SLOTS * C), np.float16)
    xpad = np.zeros((NW64 * WIN, C), np.float16)
    xpad[:N] = x.astype(np.float16)
    for c in range(NCORES):
        for j in range(SLOTS):
            for h in range(2):
                w = slot_wins[c, j, h]
                if w < 0:
                    continue
                rows = xpad[w * WIN : (w + 1) * WIN]
                xwin_pre[c, h * WIN : (h + 1) * WIN, j * C : (j + 1) * C] = rows

    return dict(
        cap=cap,
        T=T,
        slot_wins=slot_wins,
        msgs_pre=msgs_pre,
        drelw=drelw,
        xwin_pre=xwin_pre,
    )


# --------------------------------------------------------------------------
# Device program
# --------------------------------------------------------------------------
def _build_program(cap, trivial_affine):
    from contextlib import ExitStack

    import concourse.mybir as mybir
    import concourse.tile as tile
    from concourse import bacc

    f32 = mybir.dt.float32
    f16 = mybir.dt.float16
    f8e3 = mybir.dt.float8e3
    Alu = mybir.AluOpType
    Act = mybir.ActivationFunctionType
    Ax = mybir.AxisListType

    T = int(cap.sum())
    # per-tile schedule: (slot, half, first/last flag of its psum GROUP)
    tsched = []
    for j in range(SLOTS):
        for h in range(2):
            for k in range(int(cap[j, h])):
                first = (j % GRP == 0) and h == 0 and k == 0
                last = (
                    (j % GRP == GRP - 1 or j == SLOTS - 1)
                    and h == 1
                    and k == int(cap[j, 1]) - 1
                )
                tsched.append((j, h, first, last))
    assert len(tsched) == T

    nc = bacc.Bacc(
        "TRN2",
        target_bir_lowering=False,
        debug=False,
        num_devices=NCORES,
        num_swdge_queues=4,
    )

    ms_d = nc.dram_tensor("msgs", [P, T * C], f8e3, kind="ExternalInput")
    dr_d = nc.dram_tensor("drelw", [P, T], f16, kind="ExternalInput")
    io_d = nc.dram_tensor("iota3", [P, WIN * CH], f16, kind="ExternalInput")
    xw_d = nc.dram_tensor("xwin", [P, SLOTS * C], f16, kind="ExternalInput")
    w_d = nc.dram_tensor("w", [C, C], f16, kind="ExternalInput")
    br_d = nc.dram_tensor("brow", [1, GRP * C], f16, kind="ExternalInput")
    if not trivial_affine:
        gm_d = nc.dram_tensor("gm4", [P, GRP * C], f32, kind="ExternalInput")
        bt_d = nc.dram_tensor("bt4", [P, GRP * C], f32, kind="ExternalInput")
    out_d = nc.dram_tensor("out", [P, SLOTS * C], f16, kind="ExternalOutput")

    NGRP = (SLOTS + GRP - 1) // GRP

    with tile.TileContext(nc) as tc, ExitStack() as ctx:
        const = ctx.enter_context(tc.tile_pool(name="const", bufs=1))
        W_t = const.tile([C, C], f16)
        nc.sync.dma_start(W_t[:], w_d.ap())
        br4_t = const.tile([1, GRP * C], f16)
        nc.sync.dma_start(br4_t[:], br_d.ap())
        ones1 = const.tile([1, P], f16)
        nc.gpsimd.memset(ones1[:], 1.0)
        if not trivial_affine:
            gm_t = const.tile([P, GRP * C], f32)
            nc.sync.dma_start(gm_t[:], gm_d.ap())
            bt_t = const.tile([P, GRP * C], f32)
            nc.sync.dma_start(bt_t[:], bt_d.ap())
        drw_t = const.tile([P, T], f16)
        nc.sync.dma_start(drw_t[:], dr_d.ap())
        io_t = const.tile([P, WIN, CH], f16)
        nc.sync.dma_start(io_t[:], io_d.ap())
        # whole residual-x block + whole output staging block stay in SBUF
        xw_t = const.tile([P, SLOTS * C], f16)
        nc.scalar.dma_start(xw_t[:], xw_d.ap())
        ost = const.tile([P, SLOTS * C], f16)

        mpool = ctx.enter_context(tc.tile_pool(name="msgs", bufs=4))
        spool = ctx.enter_context(tc.tile_pool(name="sgen", bufs=4))
        psumA = ctx.enter_context(tc.tile_pool(name="psA", bufs=3, space="PSUM"))
        psumB = ctx.enter_context(tc.tile_pool(name="psB", bufs=2, space="PSUM"))
        epool = ctx.enter_context(tc.tile_pool(name="ep", bufs=4))
        stat = ctx.enter_context(tc.tile_pool(name="stat", bufs=4))

        def group_copy(g, pg, ng):
            """Emit right when the group's last tile matmul is queued: the
            Scalar engine drains the PSUM group into SBUF fp16 while the PE
            streams the next group's tiles."""
            nw = ng * P
            aggT16 = epool.tile([C, GRP * P], f16, tag="aggT", name=f"aggT_{g}")
            nc.scalar.activation(aggT16[:, :nw], pg[:, : ng, :, :], Act.Copy)
            return aggT16

        def pair_epilogue(entries):
            """Epilogue for up to 2 agg groups (<=8 virtual windows) batched
            into one 2-bank PSUM ps2 tile: halves the per-batch fixed costs
            on Vector/Scalar."""
            g0 = entries[0][0]
            ngt = sum(e[2] for e in entries)
            ps2 = psumB.tile([P, 2 * GRP, C], f32, tag="ps2", name=f"ps2_{g0}")
            k = 0
            for g, aggT16, ng in entries:
                for kk in range(ng):
                    nc.tensor.matmul(
                        ps2[:, k, :],
                        lhsT=aggT16[:, kk * P : (kk + 1) * P],
                        rhs=W_t[:],
                        start=(k % GRP == 0),  # first write of each PSUM bank
                        stop=False,
                        skip_group_check=True,
                    )
                    k += 1
            nc.tensor.matmul(
                ps2[:, : min(GRP, ngt), :],
                lhsT=ones1[:],
                rhs=br4_t[:, : min(GRP, ngt) * C],
                start=False,
                stop=(ngt <= GRP),
                skip_group_check=True,
            )
            if ngt > GRP:
                nc.tensor.matmul(
                    ps2[:, GRP:ngt, :],
                    lhsT=ones1[:],
                    rhs=br4_t[:, : (ngt - GRP) * C],
                    start=False,
                    stop=True,
                    skip_group_check=True,
                )
            # LN stats: sum and sum-of-squares per window (3D strided reduce)
            sum4 = stat.tile([P, 2 * GRP], f32, tag="sum", name=f"sum_{g0}")
            nc.vector.tensor_reduce(
                out=sum4[:, :ngt], in_=ps2[:, :ngt, :], axis=Ax.X, op=Alu.add
            )
            sq16 = epool.tile([P, 2 * GRP, C], f16, tag="sq", name=f"sq_{g0}")
            nc.scalar.activation(
                sq16[:, :ngt, :], ps2[:, :ngt, :], Act.Square
            )
            sqs4 = stat.tile([P, 2 * GRP], f32, tag="sqs", name=f"sqs_{g0}")
            nc.vector.tensor_reduce(
                out=sqs4[:, :ngt], in_=sq16[:, :ngt, :], axis=Ax.X, op=Alu.add
            )
            negmu = stat.tile([P, 2 * GRP], f32, tag="nmu", name=f"nmu_{g0}")
            nc.vector.tensor_scalar(
                out=negmu[:, :ngt], in0=sum4[:, :ngt], scalar1=-1.0 / C,
                scalar2=None, op0=Alu.mult,
            )
            mu2e = stat.tile([P, 2 * GRP], f32, tag="mu2", name=f"mu2_{g0}")
            nc.vector.tensor_tensor(
                out=mu2e[:, :ngt], in0=negmu[:, :ngt], in1=negmu[:, :ngt],
                op=Alu.mult,
            )
            vare = stat.tile([P, 2 * GRP], f32, tag="var", name=f"var_{g0}")
            nc.vector.tensor_scalar(
                out=vare[:, :ngt], in0=sqs4[:, :ngt], scalar1=1.0 / C,
                scalar2=LN_EPS, op0=Alu.mult, op1=Alu.add,
            )
            v2 = stat.tile([P, 2 * GRP], f32, tag="v2", name=f"v2_{g0}")
            nc.vector.tensor_tensor(
                out=v2[:, :ngt], in0=vare[:, :ngt], in1=mu2e[:, :ngt],
                op=Alu.subtract,
            )
            sd = stat.tile([P, 2 * GRP], f32, tag="sd", name=f"sd_{g0}")
            nc.scalar.activation(sd[:, :ngt], v2[:, :ngt], Act.Sqrt)
            rstd = stat.tile([P, 2 * GRP], f32, tag="rstd", name=f"rstd_{g0}")
            nc.vector.reciprocal(rstd[:, :ngt], sd[:, :ngt])
            nmr = stat.tile([P, 2 * GRP], f32, tag="nmr", name=f"nmr_{g0}")
            nc.vector.tensor_tensor(
                out=nmr[:, :ngt], in0=negmu[:, :ngt], in1=rstd[:, :ngt],
                op=Alu.mult,
            )
            # per-window fused LN affine z = t*rstd - mu*rstd (Scalar engine)
            z4 = epool.tile([P, 2 * GRP, C], f16, tag="z4", name=f"z4_{g0}")
            for k in range(ngt):
                nc.scalar.activation(
                    z4[:, k, :], ps2[:, k, :], Act.Identity,
                    bias=nmr[:, k : k + 1], scale=rstd[:, k : k + 1],
                )
            if not trivial_affine:
                y4 = epool.tile([P, 2 * GRP, C], f16, tag="y4g", name=f"y4g_{g0}")
                nc.vector.tensor_tensor(
                    out=y4[:, :ngt, :], in0=z4[:, :ngt, :],
                    in1=gm_t[:, : ngt * C], op=Alu.mult,
                )
                nc.vector.tensor_tensor(
                    out=z4[:, :ngt, :], in0=y4[:, :ngt, :],
                    in1=bt_t[:, : ngt * C], op=Alu.add,
                )
            # LeakyReLU + residual: max(z, 0.01 z) + xwin
            sc4 = epool.tile([P, 2 * GRP, C], f16, tag="sc4", name=f"sc4_{g0}")
            nc.vector.tensor_scalar(
                out=sc4[:, :ngt, :], in0=z4[:, :ngt, :], scalar1=ALPHA,
                scalar2=None, op0=Alu.mult,
            )
            lr4 = epool.tile([P, 2 * GRP, C], f16, tag="lr4", name=f"lr4_{g0}")
            nc.vector.tensor_tensor(
                out=lr4[:, :ngt, :], in0=z4[:, :ngt, :], in1=sc4[:, :ngt, :],
                op=Alu.max,
            )
            j0 = g0 * GRP
            nc.vector.tensor_tensor(
                out=ost[:, j0 * C : (j0 + ngt) * C], in0=lr4[:, :ngt, :],
                in1=xw_t[:, j0 * C : (j0 + ngt) * C], op=Alu.add,
            )
            # store the finished span
            nc.sync.dma_start(
                out_d.ap()[:, j0 * C : (j0 + ngt) * C],
                ost[:, j0 * C : (j0 + ngt) * C],
            )

        cur = None
        pending = []  # [(g, aggT16, ng)] awaiting the paired epilogue
        since = 0  # tile matmuls since the last pending entry was queued

        def flush_pending():
            nonlocal pending
            if pending:
                pair_epilogue(pending)
                pending = []

        chunks = []
        c0 = 0
        ramp = CH0
        while c0 < T:
            n = min(ramp, T - c0)
            chunks.append((c0, n))
            c0 += n
            ramp = min(CH, ramp * 2)
        for c0, n in chunks:
            mt = mpool.tile([P, CH, C], f8e3, tag="m")
            nc.sync.dma_start(mt[:, :n, :], ms_d.ap()[:, c0 * C : (c0 + n) * C])
            st = spool.tile([P, WIN, CH], f16, tag="s")
            nc.vector.tensor_tensor(
                out=st[:, :, :n],
                in0=io_t[:, :, :n],
                in1=drw_t[:, None, c0 : c0 + n].broadcast_to([P, WIN, n]),
                op=Alu.is_equal,
            )
            for i in range(n):
                j, h, first, last = tsched[c0 + i]
                g = j // GRP
                if first:
                    cur = psumA.tile(
                        [C, GRP, 2, WIN], f32, tag="agg", name=f"agg{g}"
                    )
                nc.tensor.matmul(
                    cur[:, j % GRP, h, :],
                    lhsT=mt[:, i, :],
                    rhs=st[:, :, i],
                    start=first,
                    stop=last,
                    skip_group_check=True,
                )
                since += 1
                if len(pending) >= 2 and since >= EPI_DELAY:
                    flush_pending()
                if last:
                    if len(pending) >= 2:
                        flush_pending()
                    ng = min(GRP, SLOTS - g * GRP)
                    aggT16 = group_copy(g, cur, ng)
                    pending.append((g, aggT16, ng))
                    since = 0
        flush_pending()

    nc.compile()
    return nc


# --------------------------------------------------------------------------
# Entry point
# --------------------------------------------------------------------------
def kernel(x, edge_index, W, b, gamma, beta):
    x = np.ascontiguousarray(np.asarray(x, dtype=np.float32))
    W = np.ascontiguousarray(np.asarray(W, dtype=np.float32))
    b = np.asarray(b, dtype=np.float32)
    gamma = np.asarray(gamma, dtype=np.float32)
    beta = np.asarray(beta, dtype=np.float32)

    prep = _host_prep(x, edge_index)
    cap = prep["cap"]
    trivial_affine = bool(np.all(gamma == 1.0) and np.all(beta == 0.0))

    key = (tuple(cap.reshape(-1).tolist()), trivial_affine)
    if key not in _CACHE:
        _CACHE.clear()
        _CACHE[key] = _build_program(cap, trivial_affine)
    nc = _CACHE[key]

    brow = np.tile(b[None, :], (1, GRP)).astype(np.float16)
    Ws = (W / MSG_SCALE).astype(np.float16)
    iota3 = np.ascontiguousarray(
        np.broadcast_to(
            np.arange(WIN, dtype=np.float16)[None, :, None], (P, WIN, CH)
        )
    ).reshape(P, WIN * CH)
    in_maps = []
    for c in range(NCORES):
        m = {
            "msgs": prep["msgs_pre"][c],
            "drelw": prep["drelw"][c],
            "iota3": iota3,
            "xwin": prep["xwin_pre"][c],
            "w": Ws,
            "brow": brow,
        }
        if not trivial_affine:
            m["gm4"] = np.tile(gamma[None, :], (P, GRP)).astype(np.float32)
            m["bt4"] = np.tile(beta[None, :], (P, GRP)).astype(np.float32)
        in_maps.append(m)

    from concourse import bass_utils

    trace = bool(int(os.environ.get("BASS_TRACE", "0") or "0"))
    res = bass_utils.run_bass_kernel_spmd(
        nc,
        in_maps,
        core_ids=list(range(NCORES)),
        trace=trace,
        trace_cores=list(range(NCORES)) if trace else None,
    )
    global LAST_RESULT
    LAST_RESULT = res

    out = np.zeros((N, C), dtype=np.float32)
    slot_wins = prep["slot_wins"]
    for c in range(NCORES):
        oc = np.asarray(res.results[c]["out"], dtype=np.float32)  # [P, SLOTS*C]
        for j in range(SLOTS):
            for h in range(2):
                w = slot_wins[c, j, h]
                if w < 0:
                    continue
                r0 = w * WIN
                r1 = min(r0 + WIN, N)
                out[r0:r1] = oc[
                    h * WIN : h * WIN + (r1 - r0), j * C : (j + 1) * C
                ]
    return out


# revision 33
# speedup vs baseline: 1.3604x; 1.3604x over previous
"""GCNBlock (GCNConv + LayerNorm + LeakyReLU + residual) on 8 TRN2 NeuronCores.

Strategy (graph/data parallel over destination nodes, streaming device
kernel at the memory roofline):
  * 64-node destination "windows" are assigned to cores (greedy-balanced)
    and PAIRED into 49 virtual 128-row slots per core so the epilogue runs
    at full 128-partition width.
  * Host does structure/layout prep only: degrees, the edge order (grouped
    by core/slot/half, padded to 128-edge tiles), per-edge message rows
    msgs = 8*dinv[src]*dinv[dst]*x[src] in fp8e3m4 (linearity: the full
    symmetric normalization is folded into the message values; the 8x
    prescale keeps e3m4 out of its subnormal range and is divided back out
    of W), and per-tile one-hot destination matrices (fp8e3m4, exact 0/1).
    Self-loops ride along as ordinary edges.
  * Device (all FLOPs of the reference): per 128-edge tile the PE
    accumulates aggT[c, j64] += msgs[e, c]^T @ S[e, j64] into the
    [C, 4, 2, 64] PSUM group tile of 4 virtual windows; per group the
    epilogue does (aggT)^T @ (W/8) + ones^T b (bias via rank-1 matmul into
    PSUM), LN stats via E[t^2]-mu^2 (3D strided reduces), a fused
    per-window affine (t*rstd - mu*rstd) on the Scalar engine, LeakyReLU
    via max(z, 0.01 z), and the x residual — spread across Scalar, Vector
    and GpSimd so everything hides under the DMA stream.
  * Everything streams sequentially from HBM (no per-row descriptors):
    ~19.5 MB/core total traffic (fp8 messages + 64-wide fp8 one-hots).

kernel(**inputs) takes the FULL inputs and returns the FULL [N, C] output.
"""

import os

import numpy as np

N = 50000
E = 600000
C = 128
P = 128
WIN = 64  # destination window width (one-hot width)
NCORES = 8
NW64 = (N + WIN - 1) // WIN  # 782 destination windows
HALVES = (NW64 + NCORES - 1) // NCORES  # 98 windows per core
SLOTS = (HALVES + 1) // 2  # 49 virtual (paired) windows per core
GRP = 4  # virtual windows per epilogue group / PSUM bank
LN_EPS = 1e-5
ALPHA = 0.01
MSG_SCALE = 8.0  # fp8e3m4 prescale; divided back out of W
CH = 64  # tiles (of 128 edges) per streamed chunk
CH0 = 16  # small first chunk so the PE starts early
OSPAN = 4  # slots per output store
EPI_DELAY = 8  # next-group tiles issued before a group's deferred epilogue

_CACHE: dict = {}
LAST_RESULT = None


# --------------------------------------------------------------------------
# Host-side sharding / layout prep (structure only + fp8/fp16 copies)
# --------------------------------------------------------------------------
def _host_prep(x, edge_index):
    import ml_dtypes

    f8e3 = ml_dtypes.float8_e3m4

    src = np.asarray(edge_index[0], dtype=np.int64)
    dst = np.asarray(edge_index[1], dtype=np.int64)

    deg = (np.bincount(dst, minlength=N) + 1.0).astype(np.float64)
    dinv = (1.0 / np.sqrt(deg)).astype(np.float32)

    nodes = np.arange(N, dtype=np.int64)
    asrc = np.concatenate([src, nodes])
    adst = np.concatenate([dst, nodes])
    win = adst // WIN

    cnt = np.bincount(win, minlength=NW64)  # edges (incl self-loops) per window

    # greedy balanced assignment of 64-windows to cores (largest first)
    order = np.argsort(-cnt, kind="stable")
    loads = np.zeros(NCORES, np.int64)
    nwins = np.zeros(NCORES, np.int64)
    core_of_win = np.full(NW64, -1, np.int64)
    for w in order:
        cand = np.where(nwins < HALVES)[0]
        c = cand[np.argmin(loads[cand])]
        core_of_win[w] = c
        loads[c] += cnt[w]
        nwins[c] += 1

    # half assignment: windows within a core sorted by size desc; half index
    # h -> slot j = h//2, half = h%2.  Keeps per-(slot,half) max-over-cores
    # tile caps tight.
    slot_wins = np.full((NCORES, SLOTS, 2), -1, np.int64)
    half_of_win = np.zeros(NW64, np.int64)  # flat half index within core
    for c in range(NCORES):
        ws = sorted(np.where(core_of_win == c)[0], key=lambda w: -cnt[w])
        for h, w in enumerate(ws):
            slot_wins[c, h // 2, h % 2] = w
            half_of_win[w] = h

    # per-(slot,half) tile capacity (shared across cores); >=1 so empty
    # halves still get one all-zero tile (keeps PSUM regions honestly
    # written, never read-before-write)
    cap = np.ones((SLOTS, 2), np.int64)
    for c in range(NCORES):
        for j in range(SLOTS):
            for h in range(2):
                w = slot_wins[c, j, h]
                if w >= 0:
                    cap[j, h] = max(cap[j, h], (cnt[w] + P - 1) // P)
    capf = cap.reshape(-1)
    T = int(capf.sum())
    tile_off = (np.cumsum(capf) - capf).reshape(SLOTS, 2)

    # flat destination position for every augmented edge
    ecore = core_of_win[win]
    ehalf = half_of_win[win]
    key = ecore * (2 * SLOTS) + ehalf
    sidx = np.argsort(key, kind="stable")
    key_s = key[sidx]
    uniq, start = np.unique(key_s, return_index=True)
    within = np.arange(key_s.size, dtype=np.int64) - start[
        np.searchsorted(uniq, key_s)
    ]
    half_s = key_s % (2 * SLOTS)
    slot_s = half_s // 2
    h_s = half_s % 2
    tile = tile_off[slot_s, h_s] + (within >> 7)  # tile index within core
    prow = within & 127  # partition (edge slot within tile)
    core_s = key_s // (2 * SLOTS)
    drel = (adst[sidx] % WIN).astype(np.int64)

    # fp8 message rows: full symmetric normalization folded in, 8x prescale
    es = sidx  # augmented-edge ids in layout order
    mvals = x[asrc[es]] * (
        (MSG_SCALE * dinv[asrc[es]] * dinv[adst[es]])[:, None]
    )
    msgs = np.zeros((NCORES, P, T, C), f8e3)
    msgs[core_s, prow, tile] = mvals.astype(f8e3)
    msgs_pre = np.ascontiguousarray(msgs).reshape(NCORES, P, T * C)

    onehot = np.zeros((NCORES, P, T, WIN), f8e3)
    onehot[core_s, prow, tile, drel] = 1.0
    onehot_pre = np.ascontiguousarray(onehot).reshape(NCORES, P, T * WIN)

    # residual x windows: partition p of slot j = node winA*64+p (p<64)
    # or winB*64+(p-64); padded rows -> 0
    xwin_pre = np.zeros((NCORES, P, SLOTS * C), np.float16)
    xpad = np.zeros((NW64 * WIN, C), np.float16)
    xpad[:N] = x.astype(np.float16)
    for c in range(NCORES):
        for j in range(SLOTS):
            for h in range(2):
                w = slot_wins[c, j, h]
                if w < 0:
                    continue
                rows = xpad[w * WIN : (w + 1) * WIN]
                xwin_pre[c, h * WIN : (h + 1) * WIN, j * C : (j + 1) * C] = rows

    return dict(
        cap=cap,
        T=T,
        slot_wins=slot_wins,
        msgs_pre=msgs_pre,
        onehot=onehot_pre,
        xwin_pre=xwin_pre,
    )


# --------------------------------------------------------------------------
# Device program
# --------------------------------------------------------------------------
def _build_program(cap, trivial_affine):
    from contextlib import ExitStack

    import concourse.mybir as mybir
    import concourse.tile as tile
    from concourse import bacc

    f32 = mybir.dt.float32
    f16 = mybir.dt.float16
    f8e3 = mybir.dt.float8e3
    Alu = mybir.AluOpType
    Act = mybir.ActivationFunctionType
    Ax = mybir.AxisListType

    T = int(cap.sum())
    # per-tile schedule: (slot, half, first/last flag of its psum GROUP)
    tsched = []
    for j in range(SLOTS):
        for h in range(2):
            for k in range(int(cap[j, h])):
                first = (j % GRP == 0) and h == 0 and k == 0
                last = (
                    (j % GRP == GRP - 1 or j == SLOTS - 1)
                    and h == 1
                    and k == int(cap[j, 1]) - 1
                )
                tsched.append((j, h, first, last))
    assert len(tsched) == T

    nc = bacc.Bacc(
        "TRN2",
        target_bir_lowering=False,
        debug=False,
        num_devices=NCORES,
        num_swdge_queues=4,
    )

    ms_d = nc.dram_tensor("msgs", [P, T * C], f8e3, kind="ExternalInput")
    oh_d = nc.dram_tensor("onehot", [P, T * WIN], f8e3, kind="ExternalInput")
    xw_d = nc.dram_tensor("xwin", [P, SLOTS * C], f16, kind="ExternalInput")
    w_d = nc.dram_tensor("w", [C, C], f16, kind="ExternalInput")
    br_d = nc.dram_tensor("brow", [1, GRP * C], f16, kind="ExternalInput")
    if not trivial_affine:
        gm_d = nc.dram_tensor("gm4", [P, GRP * C], f32, kind="ExternalInput")
        bt_d = nc.dram_tensor("bt4", [P, GRP * C], f32, kind="ExternalInput")
    out_d = nc.dram_tensor("out", [P, SLOTS * C], f16, kind="ExternalOutput")

    NGRP = (SLOTS + GRP - 1) // GRP

    with tile.TileContext(nc) as tc, ExitStack() as ctx:
        const = ctx.enter_context(tc.tile_pool(name="const", bufs=1))
        W_t = const.tile([C, C], f16)
        nc.sync.dma_start(W_t[:], w_d.ap())
        br4_t = const.tile([1, GRP * C], f16)
        nc.sync.dma_start(br4_t[:], br_d.ap())
        ones1 = const.tile([1, P], f16)
        nc.gpsimd.memset(ones1[:], 1.0)
        if not trivial_affine:
            gm_t = const.tile([P, GRP * C], f32)
            nc.sync.dma_start(gm_t[:], gm_d.ap())
            bt_t = const.tile([P, GRP * C], f32)
            nc.sync.dma_start(bt_t[:], bt_d.ap())
        # whole residual-x block + whole output staging block stay in SBUF
        xw_t = const.tile([P, SLOTS * C], f16)
        nc.scalar.dma_start(xw_t[:], xw_d.ap())
        ost = const.tile([P, SLOTS * C], f16)

        mpool = ctx.enter_context(tc.tile_pool(name="msgs", bufs=4))
        opool = ctx.enter_context(tc.tile_pool(name="oh", bufs=4))
        psumA = ctx.enter_context(tc.tile_pool(name="psA", bufs=3, space="PSUM"))
        psumB = ctx.enter_context(tc.tile_pool(name="psB", bufs=2, space="PSUM"))
        epool = ctx.enter_context(tc.tile_pool(name="ep", bufs=4))
        stat = ctx.enter_context(tc.tile_pool(name="stat", bufs=4))

        def group_copy(g, pg, ng):
            """Emit right when the group's last tile matmul is queued: the
            Scalar engine drains the PSUM group into SBUF fp16 while the PE
            streams the next group's tiles."""
            nw = ng * P
            aggT16 = epool.tile([C, GRP * P], f16, tag="aggT", name=f"aggT_{g}")
            nc.scalar.activation(aggT16[:, :nw], pg[:, : ng, :, :], Act.Copy)
            return aggT16

        def pair_epilogue(entries):
            """Epilogue for up to 2 agg groups (<=8 virtual windows) batched
            into one 2-bank PSUM ps2 tile: halves the per-batch fixed costs
            on Vector/Scalar."""
            g0 = entries[0][0]
            ngt = sum(e[2] for e in entries)
            ps2 = psumB.tile([P, 2 * GRP, C], f32, tag="ps2", name=f"ps2_{g0}")
            k = 0
            for g, aggT16, ng in entries:
                for kk in range(ng):
                    nc.tensor.matmul(
                        ps2[:, k, :],
                        lhsT=aggT16[:, kk * P : (kk + 1) * P],
                        rhs=W_t[:],
                        start=(k % GRP == 0),  # first write of each PSUM bank
                        stop=False,
                        skip_group_check=True,
                    )
                    k += 1
            nc.tensor.matmul(
                ps2[:, : min(GRP, ngt), :],
                lhsT=ones1[:],
                rhs=br4_t[:, : min(GRP, ngt) * C],
                start=False,
                stop=(ngt <= GRP),
                skip_group_check=True,
            )
            if ngt > GRP:
                nc.tensor.matmul(
                    ps2[:, GRP:ngt, :],
                    lhsT=ones1[:],
                    rhs=br4_t[:, : (ngt - GRP) * C],
                    start=False,
                    stop=True,
                    skip_group_check=True,
                )
            # LN stats: sum and sum-of-squares per window (3D strided reduce)
            sum4 = stat.tile([P, 2 * GRP], f32, tag="sum", name=f"sum_{g0}")
            nc.vector.tensor_reduce(
                out=sum4[:, :ngt], in_=ps2[:, :ngt, :], axis=Ax.X, op=Alu.add
            )
            sq16 = epool.tile([P, 2 * GRP, C], f16, tag="sq", name=f"sq_{g0}")
            nc.scalar.activation(
                sq16[:, :ngt, :], ps2[:, :ngt, :], Act.Square
            )
            sqs4 = stat.tile([P, 2 * GRP], f32, tag="sqs", name=f"sqs_{g0}")
            nc.vector.tensor_reduce(
                out=sqs4[:, :ngt], in_=sq16[:, :ngt, :], axis=Ax.X, op=Alu.add
            )
            negmu = stat.tile([P, 2 * GRP], f32, tag="nmu", name=f"nmu_{g0}")
            nc.vector.tensor_scalar(
                out=negmu[:, :ngt], in0=sum4[:, :ngt], scalar1=-1.0 / C,
                scalar2=None, op0=Alu.mult,
            )
            mu2e = stat.tile([P, 2 * GRP], f32, tag="mu2", name=f"mu2_{g0}")
            nc.vector.tensor_tensor(
                out=mu2e[:, :ngt], in0=negmu[:, :ngt], in1=negmu[:, :ngt],
                op=Alu.mult,
            )
            vare = stat.tile([P, 2 * GRP], f32, tag="var", name=f"var_{g0}")
            nc.vector.tensor_scalar(
                out=vare[:, :ngt], in0=sqs4[:, :ngt], scalar1=1.0 / C,
                scalar2=LN_EPS, op0=Alu.mult, op1=Alu.add,
            )
            v2 = stat.tile([P, 2 * GRP], f32, tag="v2", name=f"v2_{g0}")
            nc.vector.tensor_tensor(
                out=v2[:, :ngt], in0=vare[:, :ngt], in1=mu2e[:, :ngt],
                op=Alu.subtract,
            )
            sd = stat.tile([P, 2 * GRP], f32, tag="sd", name=f"sd_{g0}")
            nc.scalar.activation(sd[:, :ngt], v2[:, :ngt], Act.Sqrt)
            rstd = stat.tile([P, 2 * GRP], f32, tag="rstd", name=f"rstd_{g0}")
            nc.vector.reciprocal(rstd[:, :ngt], sd[:, :ngt])
            nmr = stat.tile([P, 2 * GRP], f32, tag="nmr", name=f"nmr_{g0}")
            nc.vector.tensor_tensor(
                out=nmr[:, :ngt], in0=negmu[:, :ngt], in1=rstd[:, :ngt],
                op=Alu.mult,
            )
            # per-window fused LN affine z = t*rstd - mu*rstd (Scalar engine)
            z4 = epool.tile([P, 2 * GRP, C], f16, tag="z4", name=f"z4_{g0}")
            for k in range(ngt):
                nc.scalar.activation(
                    z4[:, k, :], ps2[:, k, :], Act.Identity,
                    bias=nmr[:, k : k + 1], scale=rstd[:, k : k + 1],
                )
            if not trivial_affine:
                y4 = epool.tile([P, 2 * GRP, C], f16, tag="y4g", name=f"y4g_{g0}")
                nc.vector.tensor_tensor(
                    out=y4[:, :ngt, :], in0=z4[:, :ngt, :],
                    in1=gm_t[:, : ngt * C], op=Alu.mult,
                )
                nc.vector.tensor_tensor(
                    out=z4[:, :ngt, :], in0=y4[:, :ngt, :],
                    in1=bt_t[:, : ngt * C], op=Alu.add,
                )
            # LeakyReLU + residual: max(z, 0.01 z) + xwin
            sc4 = epool.tile([P, 2 * GRP, C], f16, tag="sc4", name=f"sc4_{g0}")
            nc.vector.tensor_scalar(
                out=sc4[:, :ngt, :], in0=z4[:, :ngt, :], scalar1=ALPHA,
                scalar2=None, op0=Alu.mult,
            )
            lr4 = epool.tile([P, 2 * GRP, C], f16, tag="lr4", name=f"lr4_{g0}")
            nc.vector.tensor_tensor(
                out=lr4[:, :ngt, :], in0=z4[:, :ngt, :], in1=sc4[:, :ngt, :],
                op=Alu.max,
            )
            j0 = g0 * GRP
            nc.vector.tensor_tensor(
                out=ost[:, j0 * C : (j0 + ngt) * C], in0=lr4[:, :ngt, :],
                in1=xw_t[:, j0 * C : (j0 + ngt) * C], op=Alu.add,
            )
            # store the finished span
            nc.sync.dma_start(
                out_d.ap()[:, j0 * C : (j0 + ngt) * C],
                ost[:, j0 * C : (j0 + ngt) * C],
            )

        cur = None
        pending = []  # [(g, aggT16, ng)] awaiting the paired epilogue
        since = 0  # tile matmuls since the last pending entry was queued

        def flush_pending():
            nonlocal pending
            if pending:
                pair_epilogue(pending)
                pending = []

        chunks = []
        c0 = 0
        ramp = CH0
        while c0 < T:
            n = min(ramp, T - c0)
            chunks.append((c0, n))
            c0 += n
            ramp = min(CH, ramp * 2)
        for c0, n in chunks:
            mt = mpool.tile([P, CH, C], f8e3, tag="m")
            nc.sync.dma_start(mt[:, :n, :], ms_d.ap()[:, c0 * C : (c0 + n) * C])
            ot = opool.tile([P, CH, WIN], f8e3, tag="o")
            nc.sync.dma_start(
                ot[:, :n, :], oh_d.ap()[:, c0 * WIN : (c0 + n) * WIN]
            )
            for i in range(n):
                j, h, first, last = tsched[c0 + i]
                g = j // GRP
                if first:
                    cur = psumA.tile(
                        [C, GRP, 2, WIN], f32, tag="agg", name=f"agg{g}"
                    )
                nc.tensor.matmul(
                    cur[:, j % GRP, h, :],
                    lhsT=mt[:, i, :],
                    rhs=ot[:, i, :],
                    start=first,
                    stop=last,
                    skip_group_check=True,
                )
                since += 1
                if len(pending) >= 2 and since >= EPI_DELAY:
                    flush_pending()
                if last:
                    if len(pending) >= 2:
                        flush_pending()
                    ng = min(GRP, SLOTS - g * GRP)
                    aggT16 = group_copy(g, cur, ng)
                    pending.append((g, aggT16, ng))
                    since = 0
        flush_pending()

    nc.compile()
    return nc


# --------------------------------------------------------------------------
# Entry point
# --------------------------------------------------------------------------
def kernel(x, edge_index, W, b, gamma, beta):
    x = np.ascontiguousarray(np.asarray(x, dtype=np.float32))
    W = np.ascontiguousarray(np.asarray(W, dtype=np.float32))
    b = np.asarray(b, dtype=np.float32)
    gamma = np.asarray(gamma, dtype=np.float32)
    beta = np.asarray(beta, dtype=np.float32)

    prep = _host_prep(x, edge_index)
    cap = prep["cap"]
    trivial_affine = bool(np.all(gamma == 1.0) and np.all(beta == 0.0))

    key = (tuple(cap.reshape(-1).tolist()), trivial_affine)
    if key not in _CACHE:
        _CACHE.clear()
        _CACHE[key] = _build_program(cap, trivial_affine)
    nc = _CACHE[key]

    brow = np.tile(b[None, :], (1, GRP)).astype(np.float16)
    Ws = (W / MSG_SCALE).astype(np.float16)
    in_maps = []
    for c in range(NCORES):
        m = {
            "msgs": prep["msgs_pre"][c],
            "onehot": prep["onehot"][c],
            "xwin": prep["xwin_pre"][c],
            "w": Ws,
            "brow": brow,
        }
        if not trivial_affine:
            m["gm4"] = np.tile(gamma[None, :], (P, GRP)).astype(np.float32)
            m["bt4"] = np.tile(beta[None, :], (P, GRP)).astype(np.float32)
        in_maps.append(m)

    from concourse import bass_utils

    trace = bool(int(os.environ.get("BASS_TRACE", "0") or "0"))
    res = bass_utils.run_bass_kernel_spmd(
        nc,
        in_maps,
        core_ids=list(range(NCORES)),
        trace=trace,
        trace_cores=list(range(NCORES)) if trace else None,
    )
    global LAST_RESULT
    LAST_RESULT = res

    out = np.zeros((N, C), dtype=np.float32)
    slot_wins = prep["slot_wins"]
    for c in range(NCORES):
        oc = np.asarray(res.results[c]["out"], dtype=np.float32)  # [P, SLOTS*C]
        for j in range(SLOTS):
            for h in range(2):
                w = slot_wins[c, j, h]
                if w < 0:
                    continue
                r0 = w * WIN
                r1 = min(r0 + WIN, N)
                out[r0:r1] = oc[
                    h * WIN : h * WIN + (r1 - r0), j * C : (j + 1) * C
                ]
    return out


# revision 34
# speedup vs baseline: 1.3773x; 1.0124x over previous
"""GCNBlock (GCNConv + LayerNorm + LeakyReLU + residual) on 8 TRN2 NeuronCores.

Strategy (graph/data parallel over destination nodes, streaming device
kernel at the memory roofline):
  * 64-node destination "windows" are assigned to cores (greedy-balanced)
    and PAIRED into 49 virtual 128-row slots per core so the epilogue runs
    at full 128-partition width.
  * Host does structure/layout prep only: degrees, the edge order (grouped
    by core/slot/half, padded to 128-edge tiles), per-edge message rows
    msgs = 8*dinv[src]*dinv[dst]*x[src] in fp8e3m4 (linearity: the full
    symmetric normalization is folded into the message values; the 8x
    prescale keeps e3m4 out of its subnormal range and is divided back out
    of W), and per-tile one-hot destination matrices (fp8e3m4, exact 0/1).
    Self-loops ride along as ordinary edges.
  * Device (all FLOPs of the reference): per 128-edge tile the PE
    accumulates aggT[c, j64] += msgs[e, c]^T @ S[e, j64] into the
    [C, 4, 2, 64] PSUM group tile of 4 virtual windows; per group the
    epilogue does (aggT)^T @ (W/8) + ones^T b (bias via rank-1 matmul into
    PSUM), LN stats via E[t^2]-mu^2 (3D strided reduces), a fused
    per-window affine (t*rstd - mu*rstd) on the Scalar engine, LeakyReLU
    via max(z, 0.01 z), and the x residual — spread across Scalar, Vector
    and GpSimd so everything hides under the DMA stream.
  * Everything streams sequentially from HBM (no per-row descriptors):
    ~19.5 MB/core total traffic (fp8 messages + 64-wide fp8 one-hots).

kernel(**inputs) takes the FULL inputs and returns the FULL [N, C] output.
"""

import os

import numpy as np

N = 50000
E = 600000
C = 128
P = 128
WIN = 64  # destination window width (one-hot width)
NCORES = 8
NW64 = (N + WIN - 1) // WIN  # 782 destination windows
HALVES = (NW64 + NCORES - 1) // NCORES  # 98 windows per core
SLOTS = (HALVES + 1) // 2  # 49 virtual (paired) windows per core
GRP = 4  # virtual windows per epilogue group / PSUM bank
LN_EPS = 1e-5
ALPHA = 0.01
MSG_SCALE = 8.0  # fp8e3m4 prescale; divided back out of W
CH = 64  # tiles (of 128 edges) per streamed chunk
CH0 = 16  # small first chunk so the PE starts early
OSPAN = 4  # slots per output store
EPI_DELAY = 8  # next-group tiles issued before a group's deferred epilogue

_CACHE: dict = {}
LAST_RESULT = None


# --------------------------------------------------------------------------
# Host-side sharding / layout prep (structure only + fp8/fp16 copies)
# --------------------------------------------------------------------------
def _host_prep(x, edge_index):
    import ml_dtypes

    f8e3 = ml_dtypes.float8_e3m4

    src = np.asarray(edge_index[0], dtype=np.int64)
    dst = np.asarray(edge_index[1], dtype=np.int64)

    deg = (np.bincount(dst, minlength=N) + 1.0).astype(np.float64)
    dinv = (1.0 / np.sqrt(deg)).astype(np.float32)

    nodes = np.arange(N, dtype=np.int64)
    asrc = np.concatenate([src, nodes])
    adst = np.concatenate([dst, nodes])
    win = adst // WIN

    cnt = np.bincount(win, minlength=NW64)  # edges (incl self-loops) per window

    # greedy balanced assignment of 64-windows to cores (largest first)
    order = np.argsort(-cnt, kind="stable")
    loads = np.zeros(NCORES, np.int64)
    nwins = np.zeros(NCORES, np.int64)
    core_of_win = np.full(NW64, -1, np.int64)
    for w in order:
        cand = np.where(nwins < HALVES)[0]
        c = cand[np.argmin(loads[cand])]
        core_of_win[w] = c
        loads[c] += cnt[w]
        nwins[c] += 1

    # half assignment: windows within a core sorted by size desc; half index
    # h -> slot j = h//2, half = h%2.  Keeps per-(slot,half) max-over-cores
    # tile caps tight.
    slot_wins = np.full((NCORES, SLOTS, 2), -1, np.int64)
    half_of_win = np.zeros(NW64, np.int64)  # flat half index within core
    for c in range(NCORES):
        ws = sorted(np.where(core_of_win == c)[0], key=lambda w: -cnt[w])
        for h, w in enumerate(ws):
            slot_wins[c, h // 2, h % 2] = w
            half_of_win[w] = h

    # per-(slot,half) tile capacity (shared across cores); >=1 so empty
    # halves still get one all-zero tile (keeps PSUM regions honestly
    # written, never read-before-write)
    cap = np.ones((SLOTS, 2), np.int64)
    for c in range(NCORES):
        for j in range(SLOTS):
            for h in range(2):
                w = slot_wins[c, j, h]
                if w >= 0:
                    cap[j, h] = max(cap[j, h], (cnt[w] + P - 1) // P)
    capf = cap.reshape(-1)
    T = int(capf.sum())
    tile_off = (np.cumsum(capf) - capf).reshape(SLOTS, 2)

    # flat destination position for every augmented edge
    ecore = core_of_win[win]
    ehalf = half_of_win[win]
    key = ecore * (2 * SLOTS) + ehalf
    sidx = np.argsort(key, kind="stable")
    key_s = key[sidx]
    uniq, start = np.unique(key_s, return_index=True)
    within = np.arange(key_s.size, dtype=np.int64) - start[
        np.searchsorted(uniq, key_s)
    ]
    half_s = key_s % (2 * SLOTS)
    slot_s = half_s // 2
    h_s = half_s % 2
    tile = tile_off[slot_s, h_s] + (within >> 7)  # tile index within core
    prow = within & 127  # partition (edge slot within tile)
    core_s = key_s // (2 * SLOTS)
    drel = (adst[sidx] % WIN).astype(np.int64)

    # fp8 message rows: full symmetric normalization folded in, 8x prescale
    es = sidx  # augmented-edge ids in layout order
    mvals = x[asrc[es]] * (
        (MSG_SCALE * dinv[asrc[es]] * dinv[adst[es]])[:, None]
    )
    msgs = np.zeros((NCORES, P, T, C), f8e3)
    msgs[core_s, prow, tile] = mvals.astype(f8e3)
    msgs_pre = np.ascontiguousarray(msgs).reshape(NCORES, P, T * C)

    onehot = np.zeros((NCORES, P, T, WIN), f8e3)
    onehot[core_s, prow, tile, drel] = 1.0
    onehot_pre = np.ascontiguousarray(onehot).reshape(NCORES, P, T * WIN)

    # residual x windows: partition p of slot j = node winA*64+p (p<64)
    # or winB*64+(p-64); padded rows -> 0
    xwin_pre = np.zeros((NCORES, P, SLOTS * C), np.float16)
    xpad = np.zeros((NW64 * WIN, C), np.float16)
    xpad[:N] = x.astype(np.float16)
    for c in range(NCORES):
        for j in range(SLOTS):
            for h in range(2):
                w = slot_wins[c, j, h]
                if w < 0:
                    continue
                rows = xpad[w * WIN : (w + 1) * WIN]
                xwin_pre[c, h * WIN : (h + 1) * WIN, j * C : (j + 1) * C] = rows

    return dict(
        cap=cap,
        T=T,
        slot_wins=slot_wins,
        msgs_pre=msgs_pre,
        onehot=onehot_pre,
        xwin_pre=xwin_pre,
    )


# --------------------------------------------------------------------------
# Device program
# --------------------------------------------------------------------------
def _build_program(cap, trivial_affine):
    from contextlib import ExitStack

    import concourse.mybir as mybir
    import concourse.tile as tile
    from concourse import bacc

    f32 = mybir.dt.float32
    f16 = mybir.dt.float16
    f8e3 = mybir.dt.float8e3
    Alu = mybir.AluOpType
    Act = mybir.ActivationFunctionType
    Ax = mybir.AxisListType

    T = int(cap.sum())
    # per-tile schedule: (slot, half, first/last flag of its psum GROUP)
    tsched = []
    for j in range(SLOTS):
        for h in range(2):
            for k in range(int(cap[j, h])):
                first = (j % GRP == 0) and h == 0 and k == 0
                last = (
                    (j % GRP == GRP - 1 or j == SLOTS - 1)
                    and h == 1
                    and k == int(cap[j, 1]) - 1
                )
                tsched.append((j, h, first, last))
    assert len(tsched) == T

    nc = bacc.Bacc(
        "TRN2",
        target_bir_lowering=False,
        debug=False,
        num_devices=NCORES,
        num_swdge_queues=4,
    )

    ms_d = nc.dram_tensor("msgs", [P, T * C], f8e3, kind="ExternalInput")
    oh_d = nc.dram_tensor("onehot", [P, T * WIN], f8e3, kind="ExternalInput")
    xw_d = nc.dram_tensor("xwin", [P, SLOTS * C], f16, kind="ExternalInput")
    w_d = nc.dram_tensor("w", [C, C], f16, kind="ExternalInput")
    br_d = nc.dram_tensor("brow", [1, GRP * C], f16, kind="ExternalInput")
    if not trivial_affine:
        gm_d = nc.dram_tensor("gm4", [P, GRP * C], f32, kind="ExternalInput")
        bt_d = nc.dram_tensor("bt4", [P, GRP * C], f32, kind="ExternalInput")
    out_d = nc.dram_tensor("out", [P, SLOTS * C], f16, kind="ExternalOutput")

    NGRP = (SLOTS + GRP - 1) // GRP

    with tile.TileContext(nc) as tc, ExitStack() as ctx:
        const = ctx.enter_context(tc.tile_pool(name="const", bufs=1))
        W_t = const.tile([C, C], f16)
        nc.sync.dma_start(W_t[:], w_d.ap())
        br4_t = const.tile([1, GRP * C], f16)
        nc.sync.dma_start(br4_t[:], br_d.ap())
        ones1 = const.tile([1, P], f16)
        nc.gpsimd.memset(ones1[:], 1.0)
        if not trivial_affine:
            gm_t = const.tile([P, GRP * C], f32)
            nc.sync.dma_start(gm_t[:], gm_d.ap())
            bt_t = const.tile([P, GRP * C], f32)
            nc.sync.dma_start(bt_t[:], bt_d.ap())
        # whole residual-x block + whole output staging block stay in SBUF
        xw_t = const.tile([P, SLOTS * C], f16)
        nc.scalar.dma_start(xw_t[:], xw_d.ap())
        ost = const.tile([P, SLOTS * C], f16)

        mpool = ctx.enter_context(tc.tile_pool(name="msgs", bufs=4))
        opool = ctx.enter_context(tc.tile_pool(name="oh", bufs=4))
        psumA = ctx.enter_context(tc.tile_pool(name="psA", bufs=3, space="PSUM"))
        psumB = ctx.enter_context(tc.tile_pool(name="psB", bufs=2, space="PSUM"))
        epool = ctx.enter_context(tc.tile_pool(name="ep", bufs=4))
        stat = ctx.enter_context(tc.tile_pool(name="stat", bufs=4))

        def group_copy(g, pg, ng):
            """Emit right when the group's last tile matmul is queued: the
            Scalar engine drains the PSUM group into SBUF fp16 while the PE
            streams the next group's tiles."""
            nw = ng * P
            aggT16 = epool.tile([C, GRP * P], f16, tag="aggT", name=f"aggT_{g}")
            nc.scalar.activation(aggT16[:, :nw], pg[:, : ng, :, :], Act.Copy)
            return aggT16

        def pair_epilogue(entries):
            """Epilogue for up to 2 agg groups (<=8 virtual windows) batched
            into one 2-bank PSUM ps2 tile: halves the per-batch fixed costs
            on Vector/Scalar."""
            g0 = entries[0][0]
            ngt = sum(e[2] for e in entries)
            ps2 = psumB.tile([P, 2 * GRP, C], f32, tag="ps2", name=f"ps2_{g0}")
            k = 0
            for g, aggT16, ng in entries:
                for kk in range(ng):
                    nc.tensor.matmul(
                        ps2[:, k, :],
                        lhsT=aggT16[:, kk * P : (kk + 1) * P],
                        rhs=W_t[:],
                        start=(k % GRP == 0),  # first write of each PSUM bank
                        stop=False,
                        skip_group_check=True,
                    )
                    k += 1
            nc.tensor.matmul(
                ps2[:, : min(GRP, ngt), :],
                lhsT=ones1[:],
                rhs=br4_t[:, : min(GRP, ngt) * C],
                start=False,
                stop=(ngt <= GRP),
                skip_group_check=True,
            )
            if ngt > GRP:
                nc.tensor.matmul(
                    ps2[:, GRP:ngt, :],
                    lhsT=ones1[:],
                    rhs=br4_t[:, : (ngt - GRP) * C],
                    start=False,
                    stop=True,
                    skip_group_check=True,
                )
            # LN stats: sum and sum-of-squares per window (3D strided reduce)
            sum4 = stat.tile([P, 2 * GRP], f32, tag="sum", name=f"sum_{g0}")
            nc.vector.tensor_reduce(
                out=sum4[:, :ngt], in_=ps2[:, :ngt, :], axis=Ax.X, op=Alu.add
            )
            sq16 = epool.tile([P, 2 * GRP, C], f16, tag="sq", name=f"sq_{g0}")
            nc.scalar.activation(
                sq16[:, :ngt, :], ps2[:, :ngt, :], Act.Square
            )
            sqs4 = stat.tile([P, 2 * GRP], f32, tag="sqs", name=f"sqs_{g0}")
            nc.vector.tensor_reduce(
                out=sqs4[:, :ngt], in_=sq16[:, :ngt, :], axis=Ax.X, op=Alu.add
            )
            negmu = stat.tile([P, 2 * GRP], f32, tag="nmu", name=f"nmu_{g0}")
            nc.vector.tensor_scalar(
                out=negmu[:, :ngt], in0=sum4[:, :ngt], scalar1=-1.0 / C,
                scalar2=None, op0=Alu.mult,
            )
            mu2e = stat.tile([P, 2 * GRP], f32, tag="mu2", name=f"mu2_{g0}")
            nc.vector.tensor_tensor(
                out=mu2e[:, :ngt], in0=negmu[:, :ngt], in1=negmu[:, :ngt],
                op=Alu.mult,
            )
            vare = stat.tile([P, 2 * GRP], f32, tag="var", name=f"var_{g0}")
            nc.vector.tensor_scalar(
                out=vare[:, :ngt], in0=sqs4[:, :ngt], scalar1=1.0 / C,
                scalar2=LN_EPS, op0=Alu.mult, op1=Alu.add,
            )
            v2 = stat.tile([P, 2 * GRP], f32, tag="v2", name=f"v2_{g0}")
            nc.vector.tensor_tensor(
                out=v2[:, :ngt], in0=vare[:, :ngt], in1=mu2e[:, :ngt],
                op=Alu.subtract,
            )
            sd = stat.tile([P, 2 * GRP], f32, tag="sd", name=f"sd_{g0}")
            nc.scalar.activation(sd[:, :ngt], v2[:, :ngt], Act.Sqrt)
            rstd = stat.tile([P, 2 * GRP], f32, tag="rstd", name=f"rstd_{g0}")
            nc.vector.reciprocal(rstd[:, :ngt], sd[:, :ngt])
            nmr = stat.tile([P, 2 * GRP], f32, tag="nmr", name=f"nmr_{g0}")
            nc.vector.tensor_tensor(
                out=nmr[:, :ngt], in0=negmu[:, :ngt], in1=rstd[:, :ngt],
                op=Alu.mult,
            )
            # per-window fused LN affine z = t*rstd - mu*rstd, split across
            # Scalar and Vector so the longest pipeline stage halves
            z4 = epool.tile([P, 2 * GRP, C], f16, tag="z4", name=f"z4_{g0}")
            for k in range(ngt):
                if k % 2 == 0:
                    nc.scalar.activation(
                        z4[:, k, :], ps2[:, k, :], Act.Identity,
                        bias=nmr[:, k : k + 1], scale=rstd[:, k : k + 1],
                    )
                else:
                    nc.vector.tensor_scalar(
                        out=z4[:, k, :], in0=ps2[:, k, :],
                        scalar1=rstd[:, k : k + 1], scalar2=nmr[:, k : k + 1],
                        op0=Alu.mult, op1=Alu.add,
                    )
            if not trivial_affine:
                y4 = epool.tile([P, 2 * GRP, C], f16, tag="y4g", name=f"y4g_{g0}")
                nc.vector.tensor_tensor(
                    out=y4[:, :ngt, :], in0=z4[:, :ngt, :],
                    in1=gm_t[:, : ngt * C], op=Alu.mult,
                )
                nc.vector.tensor_tensor(
                    out=z4[:, :ngt, :], in0=y4[:, :ngt, :],
                    in1=bt_t[:, : ngt * C], op=Alu.add,
                )
            # LeakyReLU + residual: max(z, 0.01 z) + xwin
            sc4 = epool.tile([P, 2 * GRP, C], f16, tag="sc4", name=f"sc4_{g0}")
            nc.vector.tensor_scalar(
                out=sc4[:, :ngt, :], in0=z4[:, :ngt, :], scalar1=ALPHA,
                scalar2=None, op0=Alu.mult,
            )
            lr4 = epool.tile([P, 2 * GRP, C], f16, tag="lr4", name=f"lr4_{g0}")
            nc.vector.tensor_tensor(
                out=lr4[:, :ngt, :], in0=z4[:, :ngt, :], in1=sc4[:, :ngt, :],
                op=Alu.max,
            )
            j0 = g0 * GRP
            nc.vector.tensor_tensor(
                out=ost[:, j0 * C : (j0 + ngt) * C], in0=lr4[:, :ngt, :],
                in1=xw_t[:, j0 * C : (j0 + ngt) * C], op=Alu.add,
            )
            # store the finished span
            nc.sync.dma_start(
                out_d.ap()[:, j0 * C : (j0 + ngt) * C],
                ost[:, j0 * C : (j0 + ngt) * C],
            )

        cur = None
        pending = []  # [(g, aggT16, ng)] awaiting the paired epilogue
        since = 0  # tile matmuls since the last pending entry was queued

        def flush_pending():
            nonlocal pending
            if pending:
                pair_epilogue(pending)
                pending = []

        chunks = []
        c0 = 0
        ramp = CH0
        while c0 < T:
            n = min(ramp, T - c0)
            chunks.append((c0, n))
            c0 += n
            ramp = min(CH, ramp * 2)
        for c0, n in chunks:
            mt = mpool.tile([P, CH, C], f8e3, tag="m")
            nc.sync.dma_start(mt[:, :n, :], ms_d.ap()[:, c0 * C : (c0 + n) * C])
            ot = opool.tile([P, CH, WIN], f8e3, tag="o")
            nc.sync.dma_start(
                ot[:, :n, :], oh_d.ap()[:, c0 * WIN : (c0 + n) * WIN]
            )
            for i in range(n):
                j, h, first, last = tsched[c0 + i]
                g = j // GRP
                if first:
                    cur = psumA.tile(
                        [C, GRP, 2, WIN], f32, tag="agg", name=f"agg{g}"
                    )
                nc.tensor.matmul(
                    cur[:, j % GRP, h, :],
                    lhsT=mt[:, i, :],
                    rhs=ot[:, i, :],
                    start=first,
                    stop=last,
                    skip_group_check=True,
                )
                since += 1
                if len(pending) >= 2 and since >= EPI_DELAY:
                    flush_pending()
                if last:
                    if len(pending) >= 2:
                        flush_pending()
                    ng = min(GRP, SLOTS - g * GRP)
                    aggT16 = group_copy(g, cur, ng)
                    pending.append((g, aggT16, ng))
                    since = 0
        flush_pending()

    nc.compile()
    return nc


# --------------------------------------------------------------------------
# Entry point
# --------------------------------------------------------------------------
def kernel(x, edge_index, W, b, gamma, beta):
    x = np.ascontiguousarray(np.asarray(x, dtype=np.float32))
    W = np.ascontiguousarray(np.asarray(W, dtype=np.float32))
    b = np.asarray(b, dtype=np.float32)
    gamma = np.asarray(gamma, dtype=np.float32)
    beta = np.asarray(beta, dtype=np.float32)

    prep = _host_prep(x, edge_index)
    cap = prep["cap"]
    trivial_affine = bool(np.all(gamma == 1.0) and np.all(beta == 0.0))

    key = (tuple(cap.reshape(-1).tolist()), trivial_affine)
    if key not in _CACHE:
        _CACHE.clear()
        _CACHE[key] = _build_program(cap, trivial_affine)
    nc = _CACHE[key]

    brow = np.tile(b[None, :], (1, GRP)).astype(np.float16)
    Ws = (W / MSG_SCALE).astype(np.float16)
    in_maps = []
    for c in range(NCORES):
        m = {
            "msgs": prep["msgs_pre"][c],
            "onehot": prep["onehot"][c],
            "xwin": prep["xwin_pre"][c],
            "w": Ws,
            "brow": brow,
        }
        if not trivial_affine:
            m["gm4"] = np.tile(gamma[None, :], (P, GRP)).astype(np.float32)
            m["bt4"] = np.tile(beta[None, :], (P, GRP)).astype(np.float32)
        in_maps.append(m)

    from concourse import bass_utils

    trace = bool(int(os.environ.get("BASS_TRACE", "0") or "0"))
    res = bass_utils.run_bass_kernel_spmd(
        nc,
        in_maps,
        core_ids=list(range(NCORES)),
        trace=trace,
        trace_cores=list(range(NCORES)) if trace else None,
    )
    global LAST_RESULT
    LAST_RESULT = res

    out = np.zeros((N, C), dtype=np.float32)
    slot_wins = prep["slot_wins"]
    for c in range(NCORES):
        oc = np.asarray(res.results[c]["out"], dtype=np.float32)  # [P, SLOTS*C]
        for j in range(SLOTS):
            for h in range(2):
                w = slot_wins[c, j, h]
                if w < 0:
                    continue
                r0 = w * WIN
                r1 = min(r0 + WIN, N)
                out[r0:r1] = oc[
                    h * WIN : h * WIN + (r1 - r0), j * C : (j + 1) * C
                ]
    return out
